# revision 5
# baseline (speedup 1.0000x reference)
"""MAE (masked autoencoder ViT) forward, data-parallel over 8 NeuronCores.

Sharding: batch 64 -> 8 images per core (data parallel, weights replicated).
Host does only index bookkeeping (argsort of noise, one-hot gather matrices,
patch extraction reshapes); every FLOP-bearing op (patch embed, 12 shared
encoder blocks @ ED=1024, decoder embed, 12 shared decoder blocks @ DD=512,
output head, masked-MSE loss partials) runs on the NeuronCores. Falls back to
CPU execution with identical math if the accelerator path fails.
"""
import os

os.environ.setdefault("XLA_FLAGS", "--xla_force_host_platform_device_count=8")
os.environ.setdefault("NEURON_CC_FLAGS", "--auto-cast none")

import numpy as np

B, PN, ED, DD, H = 64, 196, 1024, 512, 8
KEEP = int(PN * (1 - 0.75))  # 49
MASKN = PN - KEEP            # 147
EPS = 1e-5
N_CORES = 8
BL = B // N_CORES            # 8 images per core


def _pos_coding(patch_num, bed_dim):
    patch = int(patch_num ** 0.5)
    a = np.arange(patch).reshape(1, patch)
    b = np.arange(patch).reshape(1, patch)
    omega = 1 / 10000 ** (np.arange(bed_dim // 4).reshape(-1) / 256.0)
    g = np.stack(np.meshgrid(a, b)).reshape(2, 1, patch, patch)
    out1 = np.einsum('m,d->md', g[0].reshape(-1), omega)
    o1 = np.concatenate([np.sin(out1), np.cos(out1)], axis=1)
    out2 = np.einsum('m,d->md', g[1].reshape(-1), omega)
    o2 = np.concatenate([np.sin(out2), np.cos(out2)], axis=1)
    out = np.concatenate([o1, o2], axis=1)
    return np.concatenate([np.zeros([1, bed_dim]), out], axis=0).astype(np.float32)


POS_E = _pos_coding(PN, ED)  # (197, 1024)
POS_D = _pos_coding(PN, DD)  # (197, 512)

_CACHE = {}


def _build(jax, jnp, backend, bl=BL):
    """Build the pmapped per-shard forward for `backend`."""

    def _ln(x, g, b):
        mu = jnp.mean(x, axis=-1, keepdims=True)
        var = jnp.mean((x - mu) ** 2, axis=-1, keepdims=True)
        return (x - mu) * jax.lax.rsqrt(var + EPS) * g + b

    def _attn(x, qkv_w, qkv_b, w0_w, w0_b):
        Bx, N, C = x.shape
        qkv = (x @ qkv_w + qkv_b).reshape(Bx, N, 3, H, C // H).transpose(2, 0, 3, 1, 4)
        q, k, v = qkv[0], qkv[1], qkv[2]
        d = (C / H) ** (-0.5)
        att = jax.nn.softmax(jnp.einsum('bhnd,bhmd->bhnm', q, k) / d, axis=-1)
        o = jnp.einsum('bhnm,bhmd->bhnd', att, v).transpose(0, 2, 1, 3).reshape(Bx, N, C)
        return o @ w0_w + w0_b

    def _block(x, p):
        ln_g, ln_b, qw, qb, ww, wb, m1w, m1b, m2w, m2b = p
        z = x + _attn(_ln(x, ln_g, ln_b), qw, qb, ww, wb)
        k = _ln(z, ln_g, ln_b)
        k = jax.nn.gelu(k @ m1w + m1b, approximate=False) @ m2w + m2b
        return z + k

    def fwd(patches_g, pos_g, gmat, y, mask, w):
        # patches_g: (BL,196,768) pre-gathered patch pixels (c,kh,kw order)
        # pos_g:     (BL,196,1024) pre-gathered encoder pos-emb rows
        # gmat:      (BL,196,197) one-hot decoder gather matrices
        # y:         (BL,196,768) loss target (kh,kw,c order)
        # mask:      (BL,196)
        x = jnp.einsum('bnp,dp->bnd', patches_g, w['conv_w2d']) + w['conv_b']
        x = (x - w['bn_mean']) * jax.lax.rsqrt(w['bn_var'] + EPS) * w['bn_g'] + w['bn_b']
        x = x + pos_g
        cls = jnp.broadcast_to(w['cls_full'], (bl, 1, ED))
        tok = jnp.concatenate([cls, x], axis=1)  # (BL,197,ED)
        enc_p = (w['ln_g_e'], w['ln_b_e'], w['qkv_w_e'], w['qkv_b_e'], w['w0_w_e'],
                 w['w0_b_e'], w['mlp_w1_e'], w['mlp_b1_e'], w['mlp_w2_e'], w['mlp_b2_e'])
        tok = jax.lax.fori_loop(0, 12, lambda i, t: _block(t, enc_p), tok)
        x_mask = tok
        xd = x_mask @ w['de_w'] + w['de_b']  # (BL,197,DD)
        allp_g = jnp.einsum('bij,bjd->bid', gmat, xd)  # gather == one-hot matmul
        allp = jnp.concatenate([xd[:, :1, :], allp_g], axis=1) + w['pos_d']
        dec_p = (w['ln_g_d'], w['ln_b_d'], w['qkv_w_d'], w['qkv_b_d'], w['w0_w_d'],
                 w['w0_b_d'], w['mlp_w1_d'], w['mlp_b1_d'], w['mlp_w2_d'], w['mlp_b2_d'])
        allp = jax.lax.fori_loop(0, 12, lambda i, t: _block(t, dec_p), allp)
        allp = (allp @ w['lin_w'] + w['lin_b'])[:, 1:, :]  # (BL,196,768)
        m = mask[:, :, None]
        loss_part = jnp.sum((allp * m - y * m) ** 2, dtype=jnp.float32)
        return loss_part, x_mask, allp

    if backend == 'cpu_jit':
        return jax.jit(fwd, backend='cpu')
    return jax.pmap(fwd, in_axes=(0, 0, 0, 0, 0, None), backend=backend)


def _prep(inputs):
    """Host-side bookkeeping: reshapes, argsorts, gathers, weight dict."""
    imgs = np.asarray(inputs['imgs'], dtype=np.float32)
    noise = np.asarray(inputs['noise'], dtype=np.float32)
    patches = imgs.reshape(B, 3, 14, 16, 14, 16).transpose(0, 2, 4, 1, 3, 5).reshape(B, PN, 768)
    y = np.einsum('abcdef->acedfb', imgs.reshape(B, 3, 14, 16, 14, 16)).reshape(B, PN, 768)
    ids_shuffle = np.argsort(noise, axis=1, kind='stable')
    ids_restore = np.argsort(ids_shuffle, axis=1, kind='stable')
    mask_base = np.concatenate(
        [np.zeros((B, KEEP), np.float32), np.ones((B, MASKN), np.float32)], axis=1)
    mask = np.take_along_axis(mask_base, ids_restore, axis=1)
    patches_g = np.take_along_axis(patches, ids_restore[:, :, None], axis=1)
    pos_g = POS_E[1:][ids_restore]  # (B,196,1024)
    gmat = np.zeros((B, PN, PN + 1), np.float32)
    bidx = np.repeat(np.arange(B), PN)
    gmat[bidx, np.tile(np.arange(PN), B), (1 + ids_restore).ravel()] = 1.0

    w = {k: np.asarray(inputs[k], dtype=np.float32) for k in (
        'conv_b', 'bn_g', 'bn_b', 'bn_mean', 'bn_var',
        'qkv_w_e', 'qkv_b_e', 'w0_w_e', 'w0_b_e', 'mlp_w1_e', 'mlp_b1_e',
        'mlp_w2_e', 'mlp_b2_e', 'ln_g_e', 'ln_b_e', 'de_w', 'de_b',
        'qkv_w_d', 'qkv_b_d', 'w0_w_d', 'w0_b_d', 'mlp_w1_d', 'mlp_b1_d',
        'mlp_w2_d', 'mlp_b2_d', 'ln_g_d', 'ln_b_d', 'lin_w', 'lin_b')}
    w['conv_w2d'] = np.asarray(inputs['conv_w'], np.float32).reshape(ED, -1)
    w['cls_full'] = (np.asarray(inputs['cls_token'], np.float32)
                     + POS_E[:1][None]).reshape(1, ED)
    w['pos_d'] = POS_D[None]  # (1,197,512)

    def shard(a):
        return a.reshape(N_CORES, BL, *a.shape[1:])

    return (shard(patches_g), shard(pos_g), shard(gmat), shard(y), shard(mask)), w, mask, y


def _run(backend, sharded, w):
    import jax
    import jax.numpy as jnp
    if backend == 'cpu_jit':
        key = ('pm', backend)
        if key not in _CACHE:
            _CACHE[key] = _build(jax, jnp, backend, bl=B)
        full = tuple(a.reshape(-1, *a.shape[2:]) for a in sharded)
        loss_sum, x_mask, allp = _CACHE[key](*full, w)
        x_mask = np.asarray(x_mask)
        allp = np.asarray(allp)
        loss = np.float32(np.float64(loss_sum) / (B * PN * 768))
        return loss, x_mask, allp
    key = ('pm', backend)
    if key not in _CACHE:
        _CACHE[key] = _build(jax, jnp, backend)
    pm = _CACHE[key]
    loss_parts, x_mask_sh, allp_sh = pm(*sharded, w)
    loss_parts = np.asarray(loss_parts)
    x_mask = np.asarray(x_mask_sh).reshape(B, PN + 1, ED)
    allp = np.asarray(allp_sh).reshape(B, PN, 768)
    loss = np.float32(loss_parts.sum(dtype=np.float64) / (B * PN * 768))
    return loss, x_mask, allp


def kernel(**inputs):
    sharded, w, _mask, _y = _prep(inputs)
    try:
        # default backend = the accelerator platform (axon -> 8 NeuronCores)
        return _run(None, sharded, w)
    except Exception:
        try:
            return _run('cpu_jit', sharded, w)
        except Exception:
            return _run_numpy(sharded, w)


def _run_numpy(sharded, w):
    """Pure-numpy last-resort fallback (identical math, fp32)."""
    from scipy.special import erf  # available in env; only used on fallback
    patches_g, pos_g, gmat, y, mask = (a.reshape(-1, *a.shape[2:]) for a in sharded)

    def ln(x, g, b):
        mu = x.mean(-1, keepdims=True)
        var = ((x - mu) ** 2).mean(-1, keepdims=True)
        return (x - mu) / np.sqrt(var + EPS) * g + b

    def softmax(x):
        x = x - x.max(-1, keepdims=True)
        e = np.exp(x)
        return e / e.sum(-1, keepdims=True)

    def attn(x, qw, qb, ww, wb):
        Bx, N, C = x.shape
        qkv = (x @ qw + qb).reshape(Bx, N, 3, H, C // H).transpose(2, 0, 3, 1, 4)
        q, k, v = qkv[0], qkv[1], qkv[2]
        d = (C / H) ** (-0.5)
        att = softmax(np.einsum('bhnd,bhmd->bhnm', q, k) / d)
        o = np.einsum('bhnm,bhmd->bhnd', att, v).transpose(0, 2, 1, 3).reshape(Bx, N, C)
        return o @ ww + wb

    def gelu(x):
        return x * 0.5 * (1.0 + erf(x / np.sqrt(2.0, dtype=np.float32)))

    def block(x, p):
        ln_g, ln_b, qw, qb, ww, wb, m1w, m1b, m2w, m2b = p
        z = x + attn(ln(x, ln_g, ln_b), qw, qb, ww, wb)
        k = ln(z, ln_g, ln_b)
        k = gelu(k @ m1w + m1b) @ m2w + m2b
        return z + k

    x = np.einsum('bnp,dp->bnd', patches_g, w['conv_w2d']) + w['conv_b']
    x = (x - w['bn_mean']) / np.sqrt(w['bn_var'] + EPS) * w['bn_g'] + w['bn_b']
    x = x + pos_g
    tok = np.concatenate([np.broadcast_to(w['cls_full'], (B, 1, ED)), x], axis=1)
    enc_p = (w['ln_g_e'], w['ln_b_e'], w['qkv_w_e'], w['qkv_b_e'], w['w0_w_e'],
             w['w0_b_e'], w['mlp_w1_e'], w['mlp_b1_e'], w['mlp_w2_e'], w['mlp_b2_e'])
    for _ in range(12):
        tok = block(tok, enc_p)
    x_mask = tok
    xd = x_mask @ w['de_w'] + w['de_b']
    allp_g = np.einsum('bij,bjd->bid', gmat, xd)
    allp = np.concatenate([xd[:, :1, :], allp_g], axis=1) + w['pos_d']
    dec_p = (w['ln_g_d'], w['ln_b_d'], w['qkv_w_d'], w['qkv_b_d'], w['w0_w_d'],
             w['w0_b_d'], w['mlp_w1_d'], w['mlp_b1_d'], w['mlp_w2_d'], w['mlp_b2_d'])
    for _ in range(12):
        allp = block(allp, dec_p)
    allp = (allp @ w['lin_w'] + w['lin_b'])[:, 1:, :]
    m = mask[:, :, None]
    loss = np.float32(np.sum((allp * m - y * m) ** 2, dtype=np.float64) / (B * PN * 768))
    return loss, x_mask.astype(np.float32), allp.astype(np.float32)
